# revision 2
# baseline (speedup 1.0000x reference)
"""MLA (CustomLlamaMLAForInfer) Trainium2 Bass kernel v3.

Sharding: hybrid batch x heads across 8 NeuronCores (core c: batch c//4,
kv-head pair c%4). Host folds the low-rank up-projections into the shared
down-projection and sums the 4 o_proj partials per batch.

v3 restructure vs v2 (all bf16; fp8 DoubleRow was measured to fail the
2e-2 gate in simulation):
  P0: fused projection for tokens 0-1023 (straight-line, as v2).
  P1: projection for tokens 1024-2047 emitted as drainable ~1.4us jobs,
      zipped into attention q-blocks 0-3 (which only need lo-half K/V) --
      kills the 34us idle+HAM-cold transition between phases.
  P2: attention q-blocks 4-7 zipped with o_proj jobs (quarter-head
      granularity so the first jobs only need wo chunks 0-1, and the last
      q-block's jobs overlap its own exps).
  Causal mask applied on DVE (PSUM add before exp) instead of PE matmuls.
  w1 chunks >= RES streamed per-block to fit attn_lo alongside proj SBUF.
"""

import numpy as np

HIDDEN = 4096
N_HEADS = 32
KV_HEADS = 8
HEAD_DIM = 128
LOW_RANK = 64
TOP_K_ROPE = 32
ROPE_THETA = 10000.0
B, S = 2, 2048
NCORES = 8
GPC = 2                       # kv heads per core
QT = 8                        # q-head tiles per core
QR = QT * HEAD_DIM            # q rows per core = 1024
W1C = QR + 64 * GPC + 64 * GPC + HEAD_DIM * GPC   # 1536 fused proj cols
KCOFF = QR                    # 1024
KROFF = QR + 64 * GPC         # 1152
VOFF = KROFF + 64 * GPC       # 1280
NKT = S // 128                # 16
HT = HIDDEN // 128            # 32
RES = 20                      # resident w1 chunks; rest streamed per blk
H2 = S // 2
QB2 = 256

# pi: within-head dim order [rope_lo(0:32), rope_hi(64:96), nope_lo(32:64), nope_hi(96:128)]
PERM = np.concatenate([np.arange(0, 32), np.arange(64, 96),
                       np.arange(32, 64), np.arange(96, 128)])


def _rope_tables(seq_len):
    inv = 1.0 / (ROPE_THETA ** (np.arange(0, HEAD_DIM, 2, dtype=np.float32) / HEAD_DIM))
    pos = np.arange(seq_len, dtype=np.float32)
    fr = np.outer(pos, inv)
    emb = np.concatenate([fr, fr], axis=-1)          # [S, 128]
    return (np.cos(emb).T.astype(np.float32),        # [128, S] rows = dims
            np.sin(emb).T.astype(np.float32))


def build_program(trace_sim=False):
    from concourse import bacc, tile, mybir
    import concourse.bass as bass

    f32 = mybir.dt.float32
    bf16 = mybir.dt.bfloat16
    F32R = mybir.dt.float32r
    MS = bass.MemorySpace
    EXP = mybir.ActivationFunctionType.Exp

    nc = bacc.Bacc("TRN2", target_bir_lowering=False, debug=False,
                   num_devices=NCORES)

    def din(name, shape, dt=bf16):
        return nc.dram_tensor(name, shape, dt, kind="ExternalInput").ap()

    hidT = din("hidT", [HIDDEN, S])
    w1 = din("w1", [HIDDEN, W1C])          # fused proj weights, pre-transposed
    wo_t = din("wo_t", [QR, HIDDEN])
    qcos = din("qcos", [128, S])
    qsin = din("qsin", [128, S])
    kcos = din("kcos", [64 * GPC, S])
    ksin = din("ksin", [64 * GPC, S])
    masks = din("masks", [128, 512])
    onesd = din("ones", [128, 1], f32)
    outp = nc.dram_tensor("out_part", [S, HIDDEN], f32, kind="ExternalOutput").ap()

    with tile.TileContext(nc, trace_sim=trace_sim) as tc:
        with tc.tile_pool(name="persist", bufs=1) as pers, \
             tc.tile_pool(name="ptp", bufs=4) as ptp, \
             tc.tile_pool(name="smp", bufs=2) as smp, \
             tc.tile_pool(name="accp", bufs=2) as accp:
            # quarter (512-token) granularity: deps are tile-granular, so
            # attention on tokens <= t must not wait on later rope writes
            qT4 = [pers.tile([128, QT, 512], bf16, tag=f"qT{_i}", name=f"qT{_i}")
                   for _i in range(4)]
            kT4 = [pers.tile([128, GPC, 512], bf16, tag=f"kT{_i}", name=f"kT{_i}")
                   for _i in range(4)]
            vT4 = [pers.tile([128, 4, GPC * HEAD_DIM], bf16,
                             tag=f"vT{_i}", name=f"vT{_i}") for _i in range(4)]
            attn_lo = pers.tile([128, QT, H2], bf16, tag="attn_lo")

            masks_sb = pers.tile([128, 512], bf16, tag="masks")
            ones_sb = pers.tile([128, 1], F32R, tag="ones")

            # ---------- attention q-block emitter (shared P1/P2) ----------
            def emit_qb(qb, psS, psOU, attn_dst, dst0, drain, lookahead):
                qh_, qc0 = qb // 2, (qb % 2) * QB2
                for h in range(QT):
                    gl = h // 4
                    pso = psOU.tile([128, 512], f32, tag="psou",
                                    name=f"psou{h}_{qb}")
                    ops = pso[:, 0:256]
                    sps = pso[0:1, 256:512]
                    acc = accp.tile([128, QB2], F32R, tag="acc")
                    npair = qb + 1
                    scps = {}

                    def emit_pair(pi, h=h, gl=gl, qb=qb, scps=scps,
                                  qh_=qh_, qc0=qc0):
                        scp = psS.tile([128, 512], f32, tag="scp",
                                       name=f"scp{h}_{qb}_{pi}")
                        for u in range(2):
                            kt = 2 * pi + u
                            nc.tensor.matmul(
                                scp[:, u * 256:u * 256 + 256],
                                kT4[kt // 4][:, gl, (kt % 4) * 128:(kt % 4) * 128 + 128],
                                qT4[qh_][:, h, qc0:qc0 + QB2],
                                start=(u == 0), stop=(u == 1))
                        if pi == qb:
                            # causal bias applied on DVE, not the PE
                            nc.vector.tensor_add(scp[:], scp[:], masks_sb[:])
                        scps[pi] = scp

                    for pi in range(min(lookahead + 1, npair)):
                        emit_pair(pi)
                    for pi in range(npair):
                        if pi + lookahead + 1 < npair:
                            emit_pair(pi + lookahead + 1)
                        scp = scps.pop(pi)
                        ptile = ptp.tile([128, 512], bf16, tag="pt")
                        nc.scalar.activation(ptile[:], scp[:], EXP)
                        # filler drained between exp and AV: the PE queue is
                        # a head-blocking FIFO, so filler must sit where the
                        # AVs would otherwise stall on the exp semaphore
                        drain()
                        for u in range(2):
                            kt = 2 * pi + u
                            nc.tensor.matmul(
                                ops,
                                vT4[kt // 4][:, kt % 4,
                                             gl * HEAD_DIM:(gl + 1) * HEAD_DIM],
                                ptile[:, u * 256:u * 256 + 256],
                                start=(kt == 0), stop=(kt == 2 * npair - 1))
                        if pi == 0:
                            nc.vector.tensor_add(acc[:], ptile[:, 0:256],
                                                 ptile[:, 256:512])
                        else:
                            ptmp = smp.tile([128, QB2], F32R, tag="ptmp")
                            nc.vector.tensor_add(ptmp[:], ptile[:, 0:256],
                                                 ptile[:, 256:512])
                            nc.vector.tensor_add(acc[:], acc[:], ptmp[:])
                    nc.tensor.matmul(sps, ones_sb[:], acc[:],
                                     start=True, stop=True)
                    rec = smp.tile([1, QB2], f32, tag="rec")
                    nc.vector.reciprocal_approx_fast(out=rec[:], in_=sps)
                    rb = smp.tile([128, QB2], f32, tag="rb")
                    nc.gpsimd.partition_broadcast(rb[:], rec[:])
                    nc.vector.tensor_mul(
                        attn_dst[:, h, dst0:dst0 + QB2], ops, rb[:])
                    yield h

            # ================= P0 + P1: fused projection =================
            with tc.tile_pool(name="psS1", bufs=1, space=MS.PSUM) as psS1, \
                 tc.tile_pool(name="psOU1", bufs=1, space=MS.PSUM) as psOU1, \
                 tc.tile_pool(name="psF", bufs=4, space=MS.PSUM) as psF, \
                 tc.tile_pool(name="psKV", bufs=2, space=MS.PSUM) as psKV, \
                 tc.tile_pool(name="w1p", bufs=1) as wp, \
                 tc.tile_pool(name="w1s", bufs=6) as wsp, \
                 tc.tile_pool(name="tabs", bufs=1) as tabs, \
                 tc.tile_pool(name="hidp", bufs=6) as hp, \
                 tc.tile_pool(name="stg", bufs=1) as stg, \
                 tc.tile_pool(name="stg1", bufs=2) as stg1:
                w1t = [None] * RES

                def get_w1(t):
                    # alternate DMA queues: single-queue throughput (~170-290
                    # GB/s) races blk0's weight consumption rate; first two
                    # chunks both on sync (scalar's queue spins up later)
                    eng = nc.sync if (t < 2 or t % 2 == 0) else nc.scalar
                    if t < RES:
                        if w1t[t] is None:
                            wt = wp.tile([128, W1C], bf16, tag=f"w1_{t}",
                                         name=f"w1_{t}")
                            eng.dma_start(wt[:], w1[t * 128:(t + 1) * 128, :])
                            w1t[t] = wt
                        return w1t[t]
                    wt = wsp.tile([128, W1C], bf16, tag="w1s", name=f"w1s_{t}")
                    eng.dma_start(wt[:], w1[t * 128:(t + 1) * 128, :])
                    return wt

                qcos_sb = tabs.tile([128, S], bf16, tag="qc")
                qsin_sb = tabs.tile([128, S], bf16, tag="qs")
                kcos_sb = tabs.tile([64 * GPC, S], bf16, tag="kc")
                ksin_sb = tabs.tile([64 * GPC, S], bf16, tag="ks")

                def load_tables():
                    nc.scalar.dma_start(qcos_sb[:], qcos)
                    nc.scalar.dma_start(qsin_sb[:], qsin)
                    nc.scalar.dma_start(kcos_sb[:], kcos)
                    nc.scalar.dma_start(ksin_sb[:], ksin)

                # ---- proj job generator: per 256-token blk, 32 mm-jobs ----
                # (12 bf16 matmuls each) + stage job; rope per 512 blockpair.
                def make_proj_jobs(blks):
                    jobs = []
                    pair_st = {}
                    for blk in blks:
                        c0 = blk * 256
                        st = {}
                        st['pair'] = pair_st.setdefault(blk // 2, {})

                        def mmjob(t, blk=blk, c0=c0, st=st):
                            if t == 0:
                                st['qp'] = [psF.tile([128, 512], f32, tag="qp",
                                                     name=f"qp{blk}_{m}")
                                            for m in range(4)]
                                st['kk'] = psKV.tile([128, 512], f32, tag="kv",
                                                     name=f"kk{blk}")
                                st['vv'] = psKV.tile([128, 512], f32, tag="kv",
                                                     name=f"vv{blk}")
                            if t % 4 == 0:
                                ht = hp.tile([128, 4, 256], bf16, tag="hid")
                                nc.gpsimd.dma_start(
                                    ht[:], hidT[t * 128:(t + 4) * 128, c0:c0 + 256]
                                    .rearrange("(t p) w -> p t w", p=128))
                                st['ht'] = ht
                            ht = st['ht']
                            j = t % 4
                            wt = get_w1(t)
                            first, last = (t == 0), (t == HT - 1)
                            qp, kk, vv = st['qp'], st['kk'], st['vv']
                            for m in range(QT):
                                half = m % 2
                                nc.tensor.matmul(
                                    qp[m // 2][:, half * 256:half * 256 + 256],
                                    wt[:, m * 128:(m + 1) * 128],
                                    ht[:, j, :],
                                    start=(first and half == 0),
                                    stop=(last and half == 1))
                            nc.tensor.matmul(
                                kk[:, 0:256], wt[:, KCOFF:KCOFF + 128],
                                ht[:, j, :], start=first, stop=False)
                            nc.tensor.matmul(
                                kk[:, 256:512], wt[:, KROFF:KROFF + 128],
                                ht[:, j, :], start=False, stop=last)
                            for sblk in range(2):
                                nc.tensor.matmul(
                                    vv[:, sblk * 256:sblk * 256 + 256],
                                    ht[:, j, sblk * 128:(sblk + 1) * 128],
                                    wt[:, VOFF:VOFF + GPC * HEAD_DIM],
                                    start=(first and sblk == 0),
                                    stop=(last and sblk == 1))

                        def stagejob(blk=blk, st=st):
                            # stage + rope this 256-token half inline: the
                            # eviction latency after a block's last matmul
                            # drops to ~6us, so the blk7->P2 seam isn't gated
                            # by a 15us end-of-pair rope chain
                            half = blk % 2
                            bp = blk // 2
                            ps = st['pair']
                            if half == 0:
                                ps['qst'] = stg.tile([128, QT, 512], bf16,
                                                     tag="qst", name=f"qst{blk}")
                                ps['qrot'] = stg.tile([128, QT, 512], bf16,
                                                      tag="qrot", name=f"qrot{blk}")
                            hc = half * 256
                            tc0 = blk * 256
                            qp, kk, vv = st['qp'], st['kk'], st['vv']
                            qst, qrot = ps['qst'], ps['qrot']
                            for m in range(QT):
                                eng = nc.vector if m % 2 else nc.scalar
                                if eng is nc.vector:
                                    eng.tensor_copy(
                                        qst[:, m, hc:hc + 256],
                                        qp[m // 2][:, (m % 2) * 256:(m % 2) * 256 + 256])
                                else:
                                    eng.copy(
                                        qst[:, m, hc:hc + 256],
                                        qp[m // 2][:, (m % 2) * 256:(m % 2) * 256 + 256])
                            qs = qst[:, :, hc:hc + 256]
                            qr = qrot[:, :, hc:hc + 256]
                            nc.sync.dma_start(qr[0:32], qs[32:64])
                            nc.sync.dma_start(qr[32:64], qs[0:32])
                            nc.sync.dma_start(qr[64:96], qs[96:128])
                            nc.sync.dma_start(qr[96:128], qs[64:96])
                            for m in range(QT):
                                qd = qT4[bp][:, m, hc:hc + 256]
                                nc.vector.tensor_mul(qd, qst[:, m, hc:hc + 256],
                                                     qcos_sb[:, tc0:tc0 + 256])
                                nc.vector.tensor_mul(qrot[:, m, hc:hc + 256],
                                                     qrot[:, m, hc:hc + 256],
                                                     qsin_sb[:, tc0:tc0 + 256])
                                nc.vector.tensor_add(qd, qd,
                                                     qrot[:, m, hc:hc + 256])
                            # k: stage to bf16 (DMA can't convert f32 PSUM),
                            # rope the kr half, partition-split into kT4
                            krst = stg1.tile([128, 256], bf16, tag="krst",
                                             name=f"krst{blk}")
                            kcst = stg1.tile([128, 256], bf16, tag="kcst",
                                             name=f"kcst{blk}")
                            nc.scalar.copy(krst[:], kk[:, 256:512])
                            nc.scalar.copy(kcst[:], kk[:, 0:256])
                            krot = stg1.tile([128, 256], bf16, tag="krot",
                                             name=f"krot{blk}")
                            nc.sync.dma_start(krot[0:32, :], krst[32:64, :])
                            nc.sync.dma_start(krot[32:64, :], krst[0:32, :])
                            nc.sync.dma_start(krot[64:96, :], krst[96:128, :])
                            nc.sync.dma_start(krot[96:128, :], krst[64:96, :])
                            kst = stg1.tile([128, 256], bf16, tag="kst",
                                            name=f"kst{blk}")
                            nc.vector.tensor_mul(kst[:], krst[:],
                                                 kcos_sb[:, tc0:tc0 + 256])
                            nc.vector.tensor_mul(krot[:], krot[:],
                                                 ksin_sb[:, tc0:tc0 + 256])
                            nc.vector.tensor_add(kst[:], kst[:], krot[:])
                            nc.sync.dma_start(kT4[bp][0:64, 0, hc:hc + 256], kst[0:64, :])
                            nc.sync.dma_start(kT4[bp][0:64, 1, hc:hc + 256], kst[64:128, :])
                            nc.sync.dma_start(kT4[bp][64:128, 0, hc:hc + 256],
                                              kcst[0:64, :])
                            nc.sync.dma_start(kT4[bp][64:128, 1, hc:hc + 256],
                                              kcst[64:128, :])
                            for sblk in range(2):
                                slot = blk * 2 + sblk
                                nc.vector.tensor_copy(
                                    vT4[slot // 4][:, slot % 4, :],
                                    vv[:, sblk * 256:sblk * 256 + 256])

                        for t in range(HT):
                            jobs.append(lambda t=t, f=mmjob: f(t))
                        jobs.append(stagejob)
                    return jobs

                # ---------------- P0: tokens 0-1023, straight ----------------
                p0jobs = make_proj_jobs(range(4))
                # prime the w1 pipeline before small DMAs queue up
                for _t in range(6):
                    get_w1(_t)
                nc.scalar.dma_start(masks_sb[:], masks)
                nc.scalar.dma_start(ones_sb[:], onesd.bitcast(F32R))
                # tables must be emitted before blk0's stagejob (job 32),
                # which now ropes inline
                load_tables()
                for j in p0jobs:
                    j()

                # -------- P1: proj tokens 1024-2047 zipped with qb 0-3 -------
                p1jobs = make_proj_jobs(range(4, 8))
                pj = {'i': 0}

                def drain_pj(n=1):
                    for _ in range(n):
                        if pj['i'] < len(p1jobs):
                            p1jobs[pj['i']]()
                            pj['i'] += 1

                drain_pj(4)   # pre-burst: PE filler while scalar queue clears
                for qb in range(4):
                    nd = 2 if qb < 3 else 1
                    for h in emit_qb(qb, psS1, psOU1, attn_lo, qb * 256,
                                     lambda nd=nd: drain_pj(nd), lookahead=0):
                        pass
                while pj['i'] < len(p1jobs):
                    drain_pj(4)

            # ============== P2: attention qb 4-7 + o_proj jobs ==============
            with tc.tile_pool(name="psS2", bufs=3, space=MS.PSUM) as psS2, \
                 tc.tile_pool(name="psOU2", bufs=2, space=MS.PSUM) as psOU2, \
                 tc.tile_pool(name="ps4", bufs=3, space=MS.PSUM) as ps4, \
                 tc.tile_pool(name="wop", bufs=1) as wop, \
                 tc.tile_pool(name="ahp", bufs=1) as ahp, \
                 tc.tile_pool(name="st4", bufs=4) as st4:
                attn_hi = ahp.tile([128, QT, H2], bf16, tag="attn_hi")
                wo_sb = [wop.tile([128, HIDDEN], bf16, tag=f"wo_{hh}",
                                  name=f"wo_{hh}") for hh in range(QT)]

                def emit_wo_dmas():
                    # deferred past qb4-h0/h1: these descriptors wait on the
                    # w1-region WAR and would head-block the queues ahead of
                    # the first exps if emitted at P2 start
                    for hh in range(QT):
                        eng = (nc.sync, nc.scalar, nc.gpsimd)[hh % 3]
                        eng.dma_start(wo_sb[hh][:],
                                      wo_t[hh * 128:(hh + 1) * 128, :])

                def attn_at(h2, T):
                    if T < 8:
                        src, o = attn_lo, T * 128
                    else:
                        src, o = attn_hi, (T - 8) * 128
                    return src[:, h2, o:o + 128]

                ojobs = []
                holders = {}

                def make_ojobs(qb, quarters):
                    # quarter-head granularity: quarter q covers h2 {2q,2q+1};
                    # the bank for (T, n) accumulates across the 4 quarters.
                    for T in range(qb * 2, (qb + 1) * 2):
                        holder = holders.setdefault(T, {})
                        for q in quarters:
                            for n in range(HIDDEN // 512):
                                def job(T=T, n=n, q=q, holder=holder):
                                    if q == 0:
                                        holder[n] = ps4.tile(
                                            [128, 512], f32, tag="ps",
                                            name=f"ps{T}_{n}")
                                    ps = holder[n]
                                    for h2 in range(2 * q, 2 * q + 2):
                                        nc.tensor.matmul(
                                            ps[:], attn_at(h2, T),
                                            wo_sb[h2][:, n * 512:(n + 1) * 512],
                                            start=(h2 == 0), stop=(h2 == QT - 1))
                                    if q == 3:
                                        osb = st4.tile([128, 512], f32, tag="osb")
                                        # scalar engine: idle at the tail,
                                        # while DVE paces the attn chain
                                        nc.scalar.copy(osb[:], ps[:])
                                        nc.sync.dma_start(
                                            outp[T * 128:(T + 1) * 128,
                                                 n * 512:(n + 1) * 512],
                                            osb[:])
                                ojobs.append(job)

                nexp = {'n': 0}

                def drain_ojobs():
                    # no drains while wo streams in: a wo-gated matmul at the
                    # PE queue head would stall everything behind it
                    nexp['n'] += 1
                    if nexp['n'] <= 12:
                        return
                    # backlog-adaptive: keep jobs in reserve as tail filler
                    nd = 3 if len(ojobs) > 100 else (2 if len(ojobs) > 30 else 1)
                    for _ in range(nd):
                        if ojobs:
                            ojobs.pop(0)()

                # qb 0-3 attn is done; their o_proj jobs drain during P2.
                # qb order 4..7: qb4/qb5 read qT4[2] (roped mid-P1), so they
                # start instantly and cover the wo DMA + blk7 rope window;
                # qb6/qb7 need qT4[3] (blk7's rope), ready by then.
                for qb in range(4):
                    make_ojobs(qb, range(4))
                for qb in (4, 5, 6, 7):
                    for h in emit_qb(qb, psS2, psOU2, attn_hi, (qb - 4) * 256,
                                     drain_ojobs, lookahead=2):
                        if qb == 4 and h == 1:
                            emit_wo_dmas()
                        if h == 1:
                            make_ojobs(qb, [0])
                        elif h == 3:
                            make_ojobs(qb, [1])
                        elif h == 5:
                            make_ojobs(qb, [2])
                        elif h == 7:
                            make_ojobs(qb, [3])
                while ojobs:
                    ojobs.pop(0)()

    nc.compile()
    return nc


def make_in_maps(hidden_states, Wq, Wkr, Wdk, Wupk, Wupv, Wo):
    """Host-side sharding + layout prep (off the measured critical path)."""
    import ml_dtypes
    bf = ml_dtypes.bfloat16
    scale = np.float32(1.0 / np.sqrt(np.float32(HEAD_DIM)))

    hidden_states = np.asarray(hidden_states, np.float32)
    Wq = np.asarray(Wq, np.float32)
    Wkr = np.asarray(Wkr, np.float32)
    Wdk = np.asarray(Wdk, np.float32)
    Wupk = np.asarray(Wupk, np.float32)
    Wupv = np.asarray(Wupv, np.float32)
    Wo = np.asarray(Wo, np.float32)

    cos_t, sin_t = _rope_tables(S)                     # [128, S], rows = dims
    sgn = np.concatenate([-np.ones(32), np.ones(32),
                          -np.ones(32), np.ones(32)]).astype(np.float32)
    qcos = (cos_t[PERM] * scale).astype(bf)
    qsin = (sin_t[PERM] * sgn[:, None] * scale).astype(bf)
    rope_rows = np.concatenate([np.arange(0, 32), np.arange(64, 96)])
    ksgn = np.concatenate([-np.ones(32), np.ones(32)]).astype(np.float32)
    kcos1 = cos_t[rope_rows]                           # [64, S]
    ksin1 = sin_t[rope_rows] * ksgn[:, None]
    kcos = np.tile(kcos1, (GPC, 1)).astype(bf)
    ksin = np.tile(ksin1, (GPC, 1)).astype(bf)

    k_idx = np.arange(128)[:, None]
    # -30 bias on future (disallowed) slots, 0 on allowed: added to scores
    q_idx = np.arange(256)[None, :]
    masks = np.concatenate(
        [np.where(q_idx >= j * 128 + k_idx, 0.0, -30.0).astype(np.float32)
         for j in range(2)],
        axis=1).astype(bf)                             # [128, 512]

    hidT = [np.ascontiguousarray(
        hidden_states[b].reshape(S, HIDDEN).T).astype(bf) for b in range(B)]

    in_maps = []
    for c in range(NCORES):
        b, g = divmod(c, 4)
        wq_rows = np.concatenate(
            [Wq[(8 * g + h) * 128:(8 * g + h) * 128 + 128][PERM]
             for h in range(QT)], axis=0)              # [1024, 4096]
        wkc = Wupk[128 * g:128 * g + 128] @ Wdk        # [128, 4096]
        wkr = Wkr[128 * g:128 * g + 128]               # [128, 4096]
        wv = Wupv[256 * g:256 * g + 256] @ Wdk         # [256, 4096]
        w1 = np.ascontiguousarray(
            np.concatenate([wq_rows, wkc, wkr, wv], axis=0).T).astype(bf)
        wo_c = np.ascontiguousarray(
            Wo[:, QR * g:QR * (g + 1)].T).astype(bf)   # [1024, 4096]
        in_maps.append({
            "hidT": hidT[b], "w1": w1, "wo_t": wo_c,
            "qcos": qcos, "qsin": qsin, "kcos": kcos, "ksin": ksin,
            "masks": masks, "ones": np.ones((128, 1), np.float32),
        })
    return in_maps


def combine_outputs(results):
    outs = []
    for b in range(B):
        o = results[4 * b]["out_part"].astype(np.float32)
        for g in range(1, 4):
            o = o + results[4 * b + g]["out_part"]
        outs.append(o)
    return np.stack(outs, axis=0).reshape(B, S, HIDDEN).astype(np.float32)


_NC_CACHE = {}


def _get_program(key=0):
    if key not in _NC_CACHE:
        _NC_CACHE[key] = build_program()
    return _NC_CACHE[key]


def kernel(hidden_states, Wq, Wkr, Wdk, Wupk, Wupv, Wo):
    from concourse.bass_utils import run_bass_kernel_spmd

    in_maps = make_in_maps(hidden_states, Wq, Wkr, Wdk, Wupk, Wupv, Wo)
    nc = _get_program()
    res = run_bass_kernel_spmd(nc, in_maps, list(range(NCORES)))
    return combine_outputs(res.results)


# revision 3
# speedup vs baseline: 1.0116x; 1.0116x over previous
"""MLA (CustomLlamaMLAForInfer) Trainium2 Bass kernel v9.

Sharding: hybrid batch x heads across 8 NeuronCores (core c: batch c//4,
kv-head pair c%4). Host folds the low-rank up-projections into the shared
down-projection (W_kc = Wupk_g @ Wdk, W_v = Wupv_g @ Wdk) and sums the 4
o_proj partials per batch (host work is off the measured critical path).

All bf16 (fp8e4m3 DoubleRow was simulated at 4-6e-2 vs the 2e-2 gate --
closed). Structure tuned for the per-engine head-blocking FIFO queues:
  P0: fused projection (q|kc|kr|v, 12 bf16 matmuls per 128-row hidden
      chunk) for tokens 0-1023, straight-line; w1 chunks >= RES streamed
      per block on alternating sync/scalar DMA queues.
  P1: projection for tokens 1024-2047 emitted as drainable ~1.4us jobs,
      zipped between each exp and its AV matmuls of attention q-blocks
      0-3 -- the PE chews proj filler while AVs wait on the scalar exp,
      killing the idle+HAM-cold phase transition.
  P2: attention q-blocks 4-7 (qb4/5 first: they read qT quarter 2, roped
      mid-P1) zipped with o_proj jobs at quarter-head granularity; wo DMA
      emission deferred past qb4-h1 so the descriptors' SBUF-WAR waits
      don't head-block the exp queue; backlog-adaptive drains keep tail
      filler.
  Rope/eviction runs inline per 256-token block (stage copies split
  scalar/vector, rotate-half via partition-swap SBUF DMAs), so a block's
  K/V/Q quarters publish ~6us after its last matmul.  q/k/v live in
  512-token quarter tiles because cross-tile deps are tile-granular.
  Causal mask applied on DVE (PSUM add before exp) instead of PE matmuls;
  softmax denominator via DVE pair-sums + one f32r ones-matmul per head.
"""

import numpy as np

HIDDEN = 4096
N_HEADS = 32
KV_HEADS = 8
HEAD_DIM = 128
LOW_RANK = 64
TOP_K_ROPE = 32
ROPE_THETA = 10000.0
B, S = 2, 2048
NCORES = 8
GPC = 2                       # kv heads per core
QT = 8                        # q-head tiles per core
QR = QT * HEAD_DIM            # q rows per core = 1024
W1C = QR + 64 * GPC + 64 * GPC + HEAD_DIM * GPC   # 1536 fused proj cols
KCOFF = QR                    # 1024
KROFF = QR + 64 * GPC         # 1152
VOFF = KROFF + 64 * GPC       # 1280
NKT = S // 128                # 16
HT = HIDDEN // 128            # 32
RES = 20                      # resident w1 chunks; rest streamed per blk
H2 = S // 2
QB2 = 256

# pi: within-head dim order [rope_lo(0:32), rope_hi(64:96), nope_lo(32:64), nope_hi(96:128)]
PERM = np.concatenate([np.arange(0, 32), np.arange(64, 96),
                       np.arange(32, 64), np.arange(96, 128)])


def _rope_tables(seq_len):
    inv = 1.0 / (ROPE_THETA ** (np.arange(0, HEAD_DIM, 2, dtype=np.float32) / HEAD_DIM))
    pos = np.arange(seq_len, dtype=np.float32)
    fr = np.outer(pos, inv)
    emb = np.concatenate([fr, fr], axis=-1)          # [S, 128]
    return (np.cos(emb).T.astype(np.float32),        # [128, S] rows = dims
            np.sin(emb).T.astype(np.float32))


def build_program(trace_sim=False):
    from concourse import bacc, tile, mybir
    import concourse.bass as bass

    f32 = mybir.dt.float32
    bf16 = mybir.dt.bfloat16
    F32R = mybir.dt.float32r
    MS = bass.MemorySpace
    EXP = mybir.ActivationFunctionType.Exp

    nc = bacc.Bacc("TRN2", target_bir_lowering=False, debug=False,
                   num_devices=NCORES)

    def din(name, shape, dt=bf16):
        return nc.dram_tensor(name, shape, dt, kind="ExternalInput").ap()

    hidT = din("hidT", [HIDDEN, S])
    w1 = din("w1", [HIDDEN, W1C])          # fused proj weights, pre-transposed
    wo_t = din("wo_t", [QR, HIDDEN])
    qcos = din("qcos", [128, S])
    qsin = din("qsin", [128, S])
    kcos = din("kcos", [64 * GPC, S])
    ksin = din("ksin", [64 * GPC, S])
    masks = din("masks", [128, 512])
    onesd = din("ones", [128, 1], f32)
    outp = nc.dram_tensor("out_part", [S, HIDDEN], f32, kind="ExternalOutput").ap()

    with tile.TileContext(nc, trace_sim=trace_sim) as tc:
        with tc.tile_pool(name="persist", bufs=1) as pers, \
             tc.tile_pool(name="ptp", bufs=4) as ptp, \
             tc.tile_pool(name="smp", bufs=2) as smp, \
             tc.tile_pool(name="accp", bufs=2) as accp:
            # quarter (512-token) granularity: deps are tile-granular, so
            # attention on tokens <= t must not wait on later rope writes
            qT4 = [pers.tile([128, QT, 512], bf16, tag=f"qT{_i}", name=f"qT{_i}")
                   for _i in range(4)]
            kT4 = [pers.tile([128, GPC, 512], bf16, tag=f"kT{_i}", name=f"kT{_i}")
                   for _i in range(4)]
            vT4 = [pers.tile([128, 4, GPC * HEAD_DIM], bf16,
                             tag=f"vT{_i}", name=f"vT{_i}") for _i in range(4)]
            attn_lo = pers.tile([128, QT, H2], bf16, tag="attn_lo")

            masks_sb = pers.tile([128, 512], bf16, tag="masks")
            ones_sb = pers.tile([128, 1], F32R, tag="ones")

            # ---------- attention q-block emitter (shared P1/P2) ----------
            def emit_qb(qb, psS, psOU, attn_dst, dst0, drain, lookahead):
                qh_, qc0 = qb // 2, (qb % 2) * QB2
                for h in range(QT):
                    gl = h // 4
                    pso = psOU.tile([128, 512], f32, tag="psou",
                                    name=f"psou{h}_{qb}")
                    ops = pso[:, 0:256]
                    sps = pso[0:1, 256:512]
                    acc = accp.tile([128, QB2], F32R, tag="acc")
                    npair = qb + 1
                    scps = {}

                    def emit_pair(pi, h=h, gl=gl, qb=qb, scps=scps,
                                  qh_=qh_, qc0=qc0):
                        scp = psS.tile([128, 512], f32, tag="scp",
                                       name=f"scp{h}_{qb}_{pi}")
                        for u in range(2):
                            kt = 2 * pi + u
                            nc.tensor.matmul(
                                scp[:, u * 256:u * 256 + 256],
                                kT4[kt // 4][:, gl, (kt % 4) * 128:(kt % 4) * 128 + 128],
                                qT4[qh_][:, h, qc0:qc0 + QB2],
                                start=(u == 0), stop=(u == 1))
                        if pi == qb:
                            # causal bias applied on DVE, not the PE
                            nc.vector.tensor_add(scp[:], scp[:], masks_sb[:])
                        scps[pi] = scp

                    for pi in range(min(lookahead + 1, npair)):
                        emit_pair(pi)
                    for pi in range(npair):
                        if pi + lookahead + 1 < npair:
                            emit_pair(pi + lookahead + 1)
                        scp = scps.pop(pi)
                        ptile = ptp.tile([128, 512], bf16, tag="pt")
                        nc.scalar.activation(ptile[:], scp[:], EXP)
                        # filler drained between exp and AV: the PE queue is
                        # a head-blocking FIFO, so filler must sit where the
                        # AVs would otherwise stall on the exp semaphore
                        drain()
                        for u in range(2):
                            kt = 2 * pi + u
                            nc.tensor.matmul(
                                ops,
                                vT4[kt // 4][:, kt % 4,
                                             gl * HEAD_DIM:(gl + 1) * HEAD_DIM],
                                ptile[:, u * 256:u * 256 + 256],
                                start=(kt == 0), stop=(kt == 2 * npair - 1))
                        if pi == 0:
                            nc.vector.tensor_add(acc[:], ptile[:, 0:256],
                                                 ptile[:, 256:512])
                        else:
                            ptmp = smp.tile([128, QB2], F32R, tag="ptmp")
                            nc.vector.tensor_add(ptmp[:], ptile[:, 0:256],
                                                 ptile[:, 256:512])
                            nc.vector.tensor_add(acc[:], acc[:], ptmp[:])
                    nc.tensor.matmul(sps, ones_sb[:], acc[:],
                                     start=True, stop=True)
                    rec = smp.tile([1, QB2], f32, tag="rec")
                    nc.vector.reciprocal_approx_fast(out=rec[:], in_=sps)
                    rb = smp.tile([128, QB2], f32, tag="rb")
                    nc.gpsimd.partition_broadcast(rb[:], rec[:])
                    nc.vector.tensor_mul(
                        attn_dst[:, h, dst0:dst0 + QB2], ops, rb[:])
                    yield h

            # ================= P0 + P1: fused projection =================
            with tc.tile_pool(name="psS1", bufs=1, space=MS.PSUM) as psS1, \
                 tc.tile_pool(name="psOU1", bufs=1, space=MS.PSUM) as psOU1, \
                 tc.tile_pool(name="psF", bufs=4, space=MS.PSUM) as psF, \
                 tc.tile_pool(name="psKV", bufs=2, space=MS.PSUM) as psKV, \
                 tc.tile_pool(name="w1p", bufs=1) as wp, \
                 tc.tile_pool(name="w1s", bufs=6) as wsp, \
                 tc.tile_pool(name="tabs", bufs=1) as tabs, \
                 tc.tile_pool(name="hidp", bufs=6) as hp, \
                 tc.tile_pool(name="stg", bufs=1) as stg, \
                 tc.tile_pool(name="stg1", bufs=2) as stg1:
                w1t = [None] * RES

                def get_w1(t):
                    # alternate DMA queues: single-queue throughput (~170-290
                    # GB/s) races blk0's weight consumption rate; first two
                    # chunks both on sync (scalar's queue spins up later)
                    eng = nc.sync if (t < 2 or t % 2 == 0) else nc.scalar
                    if t < RES:
                        if w1t[t] is None:
                            wt = wp.tile([128, W1C], bf16, tag=f"w1_{t}",
                                         name=f"w1_{t}")
                            eng.dma_start(wt[:], w1[t * 128:(t + 1) * 128, :])
                            w1t[t] = wt
                        return w1t[t]
                    wt = wsp.tile([128, W1C], bf16, tag="w1s", name=f"w1s_{t}")
                    eng.dma_start(wt[:], w1[t * 128:(t + 1) * 128, :])
                    return wt

                qcos_sb = tabs.tile([128, S], bf16, tag="qc")
                qsin_sb = tabs.tile([128, S], bf16, tag="qs")
                kcos_sb = tabs.tile([64 * GPC, S], bf16, tag="kc")
                ksin_sb = tabs.tile([64 * GPC, S], bf16, tag="ks")

                def load_tables():
                    nc.scalar.dma_start(qcos_sb[:], qcos)
                    nc.scalar.dma_start(qsin_sb[:], qsin)
                    nc.scalar.dma_start(kcos_sb[:], kcos)
                    nc.scalar.dma_start(ksin_sb[:], ksin)

                # ---- proj job generator: per 256-token blk, 32 mm-jobs ----
                # (12 bf16 matmuls each) + stage job; rope per 512 blockpair.
                def make_proj_jobs(blks):
                    jobs = []
                    pair_st = {}
                    for blk in blks:
                        c0 = blk * 256
                        st = {}
                        st['pair'] = pair_st.setdefault(blk // 2, {})

                        def mmjob(t, blk=blk, c0=c0, st=st):
                            if t == 0:
                                st['qp'] = [psF.tile([128, 512], f32, tag="qp",
                                                     name=f"qp{blk}_{m}")
                                            for m in range(4)]
                                st['kk'] = psKV.tile([128, 512], f32, tag="kv",
                                                     name=f"kk{blk}")
                                st['vv'] = psKV.tile([128, 512], f32, tag="kv",
                                                     name=f"vv{blk}")
                            if t % 4 == 0:
                                ht = hp.tile([128, 4, 256], bf16, tag="hid")
                                nc.gpsimd.dma_start(
                                    ht[:], hidT[t * 128:(t + 4) * 128, c0:c0 + 256]
                                    .rearrange("(t p) w -> p t w", p=128))
                                st['ht'] = ht
                            ht = st['ht']
                            j = t % 4
                            wt = get_w1(t)
                            first, last = (t == 0), (t == HT - 1)
                            qp, kk, vv = st['qp'], st['kk'], st['vv']
                            for m in range(QT):
                                half = m % 2
                                nc.tensor.matmul(
                                    qp[m // 2][:, half * 256:half * 256 + 256],
                                    wt[:, m * 128:(m + 1) * 128],
                                    ht[:, j, :],
                                    start=(first and half == 0),
                                    stop=(last and half == 1))
                            nc.tensor.matmul(
                                kk[:, 0:256], wt[:, KCOFF:KCOFF + 128],
                                ht[:, j, :], start=first, stop=False)
                            nc.tensor.matmul(
                                kk[:, 256:512], wt[:, KROFF:KROFF + 128],
                                ht[:, j, :], start=False, stop=last)
                            for sblk in range(2):
                                nc.tensor.matmul(
                                    vv[:, sblk * 256:sblk * 256 + 256],
                                    ht[:, j, sblk * 128:(sblk + 1) * 128],
                                    wt[:, VOFF:VOFF + GPC * HEAD_DIM],
                                    start=(first and sblk == 0),
                                    stop=(last and sblk == 1))

                        def stagejob(blk=blk, st=st):
                            # stage + rope this 256-token half inline: the
                            # eviction latency after a block's last matmul
                            # drops to ~6us, so the blk7->P2 seam isn't gated
                            # by a 15us end-of-pair rope chain
                            half = blk % 2
                            bp = blk // 2
                            ps = st['pair']
                            if half == 0:
                                ps['qst'] = stg.tile([128, QT, 512], bf16,
                                                     tag="qst", name=f"qst{blk}")
                                ps['qrot'] = stg.tile([128, QT, 512], bf16,
                                                      tag="qrot", name=f"qrot{blk}")
                            hc = half * 256
                            tc0 = blk * 256
                            qp, kk, vv = st['qp'], st['kk'], st['vv']
                            qst, qrot = ps['qst'], ps['qrot']
                            for m in range(QT):
                                eng = nc.vector if m % 2 else nc.scalar
                                if eng is nc.vector:
                                    eng.tensor_copy(
                                        qst[:, m, hc:hc + 256],
                                        qp[m // 2][:, (m % 2) * 256:(m % 2) * 256 + 256])
                                else:
                                    eng.copy(
                                        qst[:, m, hc:hc + 256],
                                        qp[m // 2][:, (m % 2) * 256:(m % 2) * 256 + 256])
                            qs = qst[:, :, hc:hc + 256]
                            qr = qrot[:, :, hc:hc + 256]
                            nc.sync.dma_start(qr[0:32], qs[32:64])
                            nc.sync.dma_start(qr[32:64], qs[0:32])
                            nc.sync.dma_start(qr[64:96], qs[96:128])
                            nc.sync.dma_start(qr[96:128], qs[64:96])
                            for m in range(QT):
                                qd = qT4[bp][:, m, hc:hc + 256]
                                nc.vector.tensor_mul(qd, qst[:, m, hc:hc + 256],
                                                     qcos_sb[:, tc0:tc0 + 256])
                                nc.vector.tensor_mul(qrot[:, m, hc:hc + 256],
                                                     qrot[:, m, hc:hc + 256],
                                                     qsin_sb[:, tc0:tc0 + 256])
                                nc.vector.tensor_add(qd, qd,
                                                     qrot[:, m, hc:hc + 256])
                            # k: stage to bf16 (DMA can't convert f32 PSUM),
                            # rope the kr half, partition-split into kT4
                            krst = stg1.tile([128, 256], bf16, tag="krst",
                                             name=f"krst{blk}")
                            kcst = stg1.tile([128, 256], bf16, tag="kcst",
                                             name=f"kcst{blk}")
                            nc.scalar.copy(krst[:], kk[:, 256:512])
                            nc.scalar.copy(kcst[:], kk[:, 0:256])
                            krot = stg1.tile([128, 256], bf16, tag="krot",
                                             name=f"krot{blk}")
                            nc.sync.dma_start(krot[0:32, :], krst[32:64, :])
                            nc.sync.dma_start(krot[32:64, :], krst[0:32, :])
                            nc.sync.dma_start(krot[64:96, :], krst[96:128, :])
                            nc.sync.dma_start(krot[96:128, :], krst[64:96, :])
                            kst = stg1.tile([128, 256], bf16, tag="kst",
                                            name=f"kst{blk}")
                            nc.vector.tensor_mul(kst[:], krst[:],
                                                 kcos_sb[:, tc0:tc0 + 256])
                            nc.vector.tensor_mul(krot[:], krot[:],
                                                 ksin_sb[:, tc0:tc0 + 256])
                            nc.vector.tensor_add(kst[:], kst[:], krot[:])
                            nc.sync.dma_start(kT4[bp][0:64, 0, hc:hc + 256], kst[0:64, :])
                            nc.sync.dma_start(kT4[bp][0:64, 1, hc:hc + 256], kst[64:128, :])
                            nc.sync.dma_start(kT4[bp][64:128, 0, hc:hc + 256],
                                              kcst[0:64, :])
                            nc.sync.dma_start(kT4[bp][64:128, 1, hc:hc + 256],
                                              kcst[64:128, :])
                            for sblk in range(2):
                                slot = blk * 2 + sblk
                                nc.vector.tensor_copy(
                                    vT4[slot // 4][:, slot % 4, :],
                                    vv[:, sblk * 256:sblk * 256 + 256])

                        for t in range(HT):
                            jobs.append(lambda t=t, f=mmjob: f(t))
                        jobs.append(stagejob)
                    return jobs

                # ---------------- P0: tokens 0-1023, straight ----------------
                p0jobs = make_proj_jobs(range(4))
                # prime the w1 pipeline before small DMAs queue up
                for _t in range(6):
                    get_w1(_t)
                nc.scalar.dma_start(masks_sb[:], masks)
                nc.scalar.dma_start(ones_sb[:], onesd.bitcast(F32R))
                # tables must be emitted before blk0's stagejob (job 32),
                # which now ropes inline
                load_tables()
                for j in p0jobs:
                    j()

                # -------- P1: proj tokens 1024-2047 zipped with qb 0-3 -------
                p1jobs = make_proj_jobs(range(4, 8))
                pj = {'i': 0}

                def drain_pj(n=1):
                    for _ in range(n):
                        if pj['i'] < len(p1jobs):
                            p1jobs[pj['i']]()
                            pj['i'] += 1

                drain_pj(4)   # pre-burst: PE filler while scalar queue clears
                for qb in range(4):
                    nd = 2 if qb < 3 else 1
                    for h in emit_qb(qb, psS1, psOU1, attn_lo, qb * 256,
                                     lambda nd=nd: drain_pj(nd), lookahead=0):
                        pass
                while pj['i'] < len(p1jobs):
                    drain_pj(4)

            # ============== P2: attention qb 4-7 + o_proj jobs ==============
            with tc.tile_pool(name="psS2", bufs=3, space=MS.PSUM) as psS2, \
                 tc.tile_pool(name="psOU2", bufs=2, space=MS.PSUM) as psOU2, \
                 tc.tile_pool(name="ps4", bufs=3, space=MS.PSUM) as ps4, \
                 tc.tile_pool(name="wop", bufs=1) as wop, \
                 tc.tile_pool(name="ahp", bufs=1) as ahp, \
                 tc.tile_pool(name="st4", bufs=4) as st4:
                attn_hi = ahp.tile([128, QT, H2], bf16, tag="attn_hi")
                wo_sb = [wop.tile([128, HIDDEN], bf16, tag=f"wo_{hh}",
                                  name=f"wo_{hh}") for hh in range(QT)]

                def emit_wo_dmas():
                    # deferred past qb4-h0/h1: these descriptors wait on the
                    # w1-region WAR and would head-block the queues ahead of
                    # the first exps if emitted at P2 start
                    for hh in range(QT):
                        eng = (nc.sync, nc.scalar, nc.gpsimd)[hh % 3]
                        eng.dma_start(wo_sb[hh][:],
                                      wo_t[hh * 128:(hh + 1) * 128, :])

                def attn_at(h2, T):
                    if T < 8:
                        src, o = attn_lo, T * 128
                    else:
                        src, o = attn_hi, (T - 8) * 128
                    return src[:, h2, o:o + 128]

                ojobs = []
                holders = {}

                def make_ojobs(qb, quarters):
                    # quarter-head granularity: quarter q covers h2 {2q,2q+1};
                    # the bank for (T, n) accumulates across the 4 quarters.
                    for T in range(qb * 2, (qb + 1) * 2):
                        holder = holders.setdefault(T, {})
                        for q in quarters:
                            for n in range(HIDDEN // 512):
                                def job(T=T, n=n, q=q, holder=holder):
                                    if q == 0:
                                        holder[n] = ps4.tile(
                                            [128, 512], f32, tag="ps",
                                            name=f"ps{T}_{n}")
                                    ps = holder[n]
                                    for h2 in range(2 * q, 2 * q + 2):
                                        nc.tensor.matmul(
                                            ps[:], attn_at(h2, T),
                                            wo_sb[h2][:, n * 512:(n + 1) * 512],
                                            start=(h2 == 0), stop=(h2 == QT - 1))
                                    if q == 3:
                                        osb = st4.tile([128, 512], f32, tag="osb")
                                        # scalar engine: idle at the tail,
                                        # while DVE paces the attn chain
                                        nc.scalar.copy(osb[:], ps[:])
                                        nc.sync.dma_start(
                                            outp[T * 128:(T + 1) * 128,
                                                 n * 512:(n + 1) * 512],
                                            osb[:])
                                ojobs.append(job)

                nexp = {'n': 0}

                def drain_ojobs():
                    # no drains while wo streams in: a wo-gated matmul at the
                    # PE queue head would stall everything behind it
                    nexp['n'] += 1
                    if nexp['n'] <= 12:
                        return
                    # backlog-adaptive: keep jobs in reserve as tail filler
                    nd = 3 if len(ojobs) > 100 else (2 if len(ojobs) > 30 else 1)
                    for _ in range(nd):
                        if ojobs:
                            ojobs.pop(0)()

                # qb 0-3 attn is done; their o_proj jobs drain during P2.
                # qb order 4..7: qb4/qb5 read qT4[2] (roped mid-P1), so they
                # start instantly and cover the wo DMA + blk7 rope window;
                # qb6/qb7 need qT4[3] (blk7's rope), ready by then.
                for qb in range(4):
                    make_ojobs(qb, range(4))
                for qb in (4, 5, 6, 7):
                    for h in emit_qb(qb, psS2, psOU2, attn_hi, (qb - 4) * 256,
                                     drain_ojobs, lookahead=2):
                        if qb == 4 and h == 1:
                            emit_wo_dmas()
                        if h == 1:
                            make_ojobs(qb, [0])
                        elif h == 3:
                            make_ojobs(qb, [1])
                        elif h == 5:
                            make_ojobs(qb, [2])
                        elif h == 7:
                            make_ojobs(qb, [3])
                while ojobs:
                    ojobs.pop(0)()

    nc.compile()
    return nc


def make_in_maps(hidden_states, Wq, Wkr, Wdk, Wupk, Wupv, Wo):
    """Host-side sharding + layout prep (off the measured critical path)."""
    import ml_dtypes
    bf = ml_dtypes.bfloat16
    scale = np.float32(1.0 / np.sqrt(np.float32(HEAD_DIM)))

    hidden_states = np.asarray(hidden_states, np.float32)
    Wq = np.asarray(Wq, np.float32)
    Wkr = np.asarray(Wkr, np.float32)
    Wdk = np.asarray(Wdk, np.float32)
    Wupk = np.asarray(Wupk, np.float32)
    Wupv = np.asarray(Wupv, np.float32)
    Wo = np.asarray(Wo, np.float32)

    cos_t, sin_t = _rope_tables(S)                     # [128, S], rows = dims
    sgn = np.concatenate([-np.ones(32), np.ones(32),
                          -np.ones(32), np.ones(32)]).astype(np.float32)
    qcos = (cos_t[PERM] * scale).astype(bf)
    qsin = (sin_t[PERM] * sgn[:, None] * scale).astype(bf)
    rope_rows = np.concatenate([np.arange(0, 32), np.arange(64, 96)])
    ksgn = np.concatenate([-np.ones(32), np.ones(32)]).astype(np.float32)
    kcos1 = cos_t[rope_rows]                           # [64, S]
    ksin1 = sin_t[rope_rows] * ksgn[:, None]
    kcos = np.tile(kcos1, (GPC, 1)).astype(bf)
    ksin = np.tile(ksin1, (GPC, 1)).astype(bf)

    k_idx = np.arange(128)[:, None]
    # -30 bias on future (disallowed) slots, 0 on allowed: added to scores
    q_idx = np.arange(256)[None, :]
    masks = np.concatenate(
        [np.where(q_idx >= j * 128 + k_idx, 0.0, -30.0).astype(np.float32)
         for j in range(2)],
        axis=1).astype(bf)                             # [128, 512]

    hidT = [np.ascontiguousarray(
        hidden_states[b].reshape(S, HIDDEN).T).astype(bf) for b in range(B)]

    in_maps = []
    for c in range(NCORES):
        b, g = divmod(c, 4)
        wq_rows = np.concatenate(
            [Wq[(8 * g + h) * 128:(8 * g + h) * 128 + 128][PERM]
             for h in range(QT)], axis=0)              # [1024, 4096]
        wkc = Wupk[128 * g:128 * g + 128] @ Wdk        # [128, 4096]
        wkr = Wkr[128 * g:128 * g + 128]               # [128, 4096]
        wv = Wupv[256 * g:256 * g + 256] @ Wdk         # [256, 4096]
        w1 = np.ascontiguousarray(
            np.concatenate([wq_rows, wkc, wkr, wv], axis=0).T).astype(bf)
        wo_c = np.ascontiguousarray(
            Wo[:, QR * g:QR * (g + 1)].T).astype(bf)   # [1024, 4096]
        in_maps.append({
            "hidT": hidT[b], "w1": w1, "wo_t": wo_c,
            "qcos": qcos, "qsin": qsin, "kcos": kcos, "ksin": ksin,
            "masks": masks, "ones": np.ones((128, 1), np.float32),
        })
    return in_maps


def combine_outputs(results):
    outs = []
    for b in range(B):
        o = results[4 * b]["out_part"].astype(np.float32)
        for g in range(1, 4):
            o = o + results[4 * b + g]["out_part"]
        outs.append(o)
    return np.stack(outs, axis=0).reshape(B, S, HIDDEN).astype(np.float32)


_NC_CACHE = {}


def _get_program(key=0):
    if key not in _NC_CACHE:
        _NC_CACHE[key] = build_program()
    return _NC_CACHE[key]


def kernel(hidden_states, Wq, Wkr, Wdk, Wupk, Wupv, Wo):
    from concourse.bass_utils import run_bass_kernel_spmd

    in_maps = make_in_maps(hidden_states, Wq, Wkr, Wdk, Wupk, Wupv, Wo)
    nc = _get_program()
    res = run_bass_kernel_spmd(nc, in_maps, list(range(NCORES)))
    return combine_outputs(res.results)
